# revision 12
# baseline (speedup 1.0000x reference)
"""DeepKMeans (vq_codebook) Trainium2 Bass kernel.

Data-parallel over the batch dim: 8192 rows split as 1024 rows on each of the
8 NeuronCores; MLP weights and cluster reps are replicated. Each core runs a
fused autoencoder + softmin-weighted-distance pipeline:

  h   = relu(x @ W1 + b1)           [1024, 256]
  emb = h @ W2 + b2                 [1024, 32]
  h2  = relu(emb @ W3 + b3)         [1024, 256]
  rec = h2 @ W4 + b4                [1024, 784]
  dist[b,k] = |emb_b - rep_k|^2     [1024, 512]
  softmin weighting with alpha=1000

On-chip layout: the encoder runs feature-major (features on partitions, batch
in the free dim) so no intermediate activation ever needs transposing; only x
itself is transposed (PE transposes). The decoder matmul and the distance
matmul consume the feature-major activations as stationary (lhsT) operands and
produce batch-major outputs, which DMA back to DRAM contiguously.

Distances use the shifted form dist - |emb|^2 = [emb; 1].T @ [-2 reps^T;
|reps|^2] (one matmul into PSUM); the softmin is invariant to the per-row
|emb|^2 shift, which is added back only when evacuating distances to SBUF
(as a per-partition activation bias).

Precision: the distance chain (mm1, mm2, mm3?, dist) runs exact fp32 on the
PE; alpha=1000 amplifies distance errors by 1000x in the softmin exponent, so
reduced-precision matmuls there risk argmin flips. The reconstruction decoder
(mm4) runs fp32r (4x faster streaming) since its error budget is loose.
"""

import sys
import numpy as np

if "/opt/trn_rl_repo" not in sys.path:
    sys.path.insert(0, "/opt/trn_rl_repo")

import concourse.bacc as bacc
import concourse.mybir as mybir
from concourse import masks, tile
from concourse.bass_utils import run_bass_kernel_spmd

FP32 = mybir.dt.float32
F32R = mybir.dt.float32r
AF = mybir.ActivationFunctionType
ALU = mybir.AluOpType
AX = mybir.AxisListType

N_CORES = 8
B, D, H, E, K = 8192, 784, 256, 32, 512
BL = B // N_CORES          # 1024 rows per core
BF = 512                   # batch rows per chunk (free dim of feature-major tiles)
NCHUNK = BL // BF          # 2
NSLICE = BF // 128         # 4 x 128-row slices per chunk
DP = 112                   # partition size of a D-chunk (784 = 7 * 112)
DC = D // DP               # 7
ALPHA = 1000.0

# dtype of the encoder/distance matmul chain and of the decoder (mm4)
ENC_DT = FP32
DEC_DT = F32R


def build_kernel(bufs_x=4, bufs_xt=2, bufs_act=2, bufs_out=4, bufs_psxt=2,
                 bufs_psmm=6):
    nc = bacc.Bacc("TRN2", target_bir_lowering=False, debug=False)

    x_d = nc.dram_tensor("x", [BL, D], FP32, kind="ExternalInput").ap()
    reps_d = nc.dram_tensor("cluster_reps", [K, E], FP32, kind="ExternalInput").ap()
    w1_d = nc.dram_tensor("W1", [D, H], FP32, kind="ExternalInput").ap()
    b1_d = nc.dram_tensor("b1", [H], FP32, kind="ExternalInput").ap()
    w2_d = nc.dram_tensor("W2", [H, E], FP32, kind="ExternalInput").ap()
    b2_d = nc.dram_tensor("b2", [E], FP32, kind="ExternalInput").ap()
    w3_d = nc.dram_tensor("W3", [E, H], FP32, kind="ExternalInput").ap()
    b3_d = nc.dram_tensor("b3", [H], FP32, kind="ExternalInput").ap()
    w4_d = nc.dram_tensor("W4", [H, D], FP32, kind="ExternalInput").ap()
    b4_d = nc.dram_tensor("b4", [D], FP32, kind="ExternalInput").ap()

    wd_d = nc.dram_tensor("weighted", [BL, K], FP32, kind="ExternalOutput").ap()
    dist_d = nc.dram_tensor("distances", [BL, K], FP32, kind="ExternalOutput").ap()
    rec_d = nc.dram_tensor("reconstruction", [BL, D], FP32, kind="ExternalOutput").ap()
    emb_d = nc.dram_tensor("embeddings", [BL, E], FP32, kind="ExternalOutput").ap()

    from contextlib import ExitStack
    with tile.TileContext(nc) as tc, ExitStack() as ctx:
        consts = ctx.enter_context(tc.tile_pool(name="consts", bufs=1))
        xpool = ctx.enter_context(tc.tile_pool(name="x", bufs=bufs_x))
        xtpool = ctx.enter_context(tc.tile_pool(name="xt", bufs=bufs_xt))
        actp = ctx.enter_context(tc.tile_pool(name="act", bufs=bufs_act))
        outp = ctx.enter_context(tc.tile_pool(name="out", bufs=bufs_out))
        stat = ctx.enter_context(tc.tile_pool(name="stat", bufs=8))
        ps_xt = ctx.enter_context(
            tc.tile_pool(name="ps_xt", bufs=bufs_psxt, space="PSUM"))
        ps_mm = ctx.enter_context(
            tc.tile_pool(name="ps_mm", bufs=bufs_psmm, space="PSUM"))

        # ---------------- one-time setup ----------------
        ident = consts.tile([128, 128], FP32)
        masks.make_identity(nc, ident[:])

        # first x chunk loads issued before the weights so compute starts early
        x_tiles = {}
        for t in range(NSLICE):
            xt = xpool.tile([128, D], FP32, tag="xin")
            nc.sync.dma_start(xt[:], x_d[t * 128:(t + 1) * 128, :])
            x_tiles[(0, t)] = xt

        w1_sb = consts.tile([DP, DC, H], ENC_DT)
        nc.sync.dma_start(w1_sb[:], w1_d.rearrange("(c p) h -> p c h", p=DP).bitcast(ENC_DT))
        w2_sb = consts.tile([128, 2, E], ENC_DT)
        nc.sync.dma_start(w2_sb[:], w2_d.rearrange("(c p) e -> p c e", p=128).bitcast(ENC_DT))
        w3_sb = consts.tile([E, H], ENC_DT)
        nc.sync.dma_start(w3_sb[:], w3_d.bitcast(ENC_DT))
        w4_sb = consts.tile([128, 2, D], DEC_DT)
        nc.sync.dma_start(w4_sb[:], w4_d.rearrange("(c p) d -> p c d", p=128).bitcast(DEC_DT))
        b1_sb = consts.tile([128, 2], FP32)
        nc.sync.dma_start(b1_sb[:], b1_d.rearrange("(c p) -> p c", p=128))
        b2_sb = consts.tile([E, 1], FP32)
        nc.sync.dma_start(b2_sb[:], b2_d.rearrange("(p c) -> p c", c=1))
        b3_sb = consts.tile([128, 2], FP32)
        nc.sync.dma_start(b3_sb[:], b3_d.rearrange("(c p) -> p c", p=128))
        b4_row = consts.tile([1, D], FP32)
        nc.sync.dma_start(b4_row[:], b4_d.rearrange("(c p) -> c p", c=1))

        ones_col = consts.tile([E, 1], FP32)
        nc.gpsimd.memset(ones_col[:], 1.0)
        ones_row = consts.tile([1, 128], FP32)
        nc.gpsimd.memset(ones_row[:], 1.0)

        # broadcast b4 across all 128 partitions: b4_bcast = ones(128,1) @ b4(1,D)
        b4_bcast = consts.tile([128, D], FP32)
        bb_ps0 = ps_mm.tile([128, 512], FP32, tag="mm")
        bb_ps1 = ps_mm.tile([128, D - 512], FP32, tag="mm")
        nc.tensor.matmul(bb_ps0[:], ones_row[:], b4_row[:, 0:512],
                         start=True, stop=True)
        nc.tensor.matmul(bb_ps1[:], ones_row[:], b4_row[:, 512:D],
                         start=True, stop=True)
        nc.scalar.copy(b4_bcast[:, 0:512], bb_ps0[:])
        nc.scalar.copy(b4_bcast[:, 512:D], bb_ps1[:])

        # cluster reps -> reps_aug [33, 512]: rows 0..31 = -2*reps^T,
        # row 32 = |reps|^2. dist - |emb|^2 = [emb; 1].T @ reps_aug.
        r_in = consts.tile([128, 4, E], FP32)
        nc.sync.dma_start(r_in[:], reps_d.rearrange("(c p) e -> p c e", p=128))
        reps_aug = consts.tile([E + 1, K], ENC_DT)
        for c in range(4):
            rt_ps = ps_mm.tile([E, 128], FP32, tag="mm")
            nc.tensor.transpose(rt_ps[:], r_in[:, c, :], ident[:])
            nc.scalar.mul(reps_aug[0:E, c * 128:(c + 1) * 128], rt_ps[:], -2.0)
        reps_sq = consts.tile([E, K], FP32)
        nc.scalar.activation(reps_sq[:], reps_aug[0:E, :].bitcast(FP32), AF.Square)
        cn_ps = ps_mm.tile([1, K], FP32, tag="mm")
        nc.tensor.matmul(cn_ps[:], ones_col[:], reps_sq[:], start=True, stop=True)
        # reps_sq holds 4*|reps|^2 (rows are -2*reps), so scale by 1/4
        nc.scalar.mul(reps_aug[E:E + 1, :], cn_ps[:], 0.25)

        # ---------------- main loop over batch chunks ----------------
        for ch in range(NCHUNK):
            row0 = ch * BF

            for t in range(NSLICE):
                if (ch, t) not in x_tiles:
                    xt = xpool.tile([128, D], FP32, tag="xin")
                    nc.sync.dma_start(
                        xt[:], x_d[row0 + t * 128:row0 + (t + 1) * 128, :])
                    x_tiles[(ch, t)] = xt

            # transpose x chunk into xT [112, 7, 4, 128] (d-part, d-chunk, slice, b)
            xT = xtpool.tile([DP, DC, NSLICE, 128], ENC_DT, tag="xT")
            for t in range(NSLICE):
                xt = x_tiles.pop((ch, t))
                ps_a = ps_xt.tile([DP, 4, 128], FP32, tag="xt")
                for c in range(4):
                    nc.tensor.transpose(
                        ps_a[:, c, :], xt[:, c * DP:(c + 1) * DP], ident[:])
                ps_b = ps_xt.tile([DP, 3, 128], FP32, tag="xt")
                for c in range(3):
                    nc.tensor.transpose(
                        ps_b[:, c, :], xt[:, (c + 4) * DP:(c + 5) * DP], ident[:])
                dst_a = xT[:, 0:4, t, :].bitcast(FP32)
                dst_b = xT[:, 4:7, t, :].bitcast(FP32)
                if t % 2 == 0:
                    nc.vector.tensor_copy(dst_a, ps_a[:])
                    nc.scalar.copy(dst_b, ps_b[:])
                else:
                    nc.scalar.copy(dst_a, ps_a[:])
                    nc.vector.tensor_copy(dst_b, ps_b[:])

            # mm1: hT[hc] [128, 512] = relu(W1.T @ x.T + b1)
            hT = []
            for hc in range(2):
                h_ps = ps_mm.tile([128, BF], FP32, tag="mm")
                for c in range(DC):
                    nc.tensor.matmul(
                        h_ps[:],
                        w1_sb[:, c, hc * 128:(hc + 1) * 128],
                        xT[:, c, :, :],
                        start=(c == 0), stop=(c == DC - 1))
                h_sb = actp.tile([128, BF], ENC_DT, tag="hT")
                nc.scalar.activation(h_sb[:].bitcast(FP32), h_ps[:], AF.Relu,
                                     bias=b1_sb[:, hc:hc + 1])
                hT.append(h_sb)

            # mm2: embT [32, 512] = W2.T @ hT + b2 ; row 32 = ones
            e_ps = ps_mm.tile([E, BF], FP32, tag="mm")
            for hc in range(2):
                nc.tensor.matmul(e_ps[:], w2_sb[:, hc, :], hT[hc][:],
                                 start=(hc == 0), stop=(hc == 1))
            emb_aug = actp.tile([E + 1, BF], ENC_DT, tag="emb")
            nc.scalar.activation(emb_aug[0:E, :].bitcast(FP32), e_ps[:],
                                 AF.Identity, bias=b2_sb[:])
            nc.gpsimd.memset(emb_aug[E:E + 1, :], 1.0)

            # embeddings out (batch-major) + per-row |emb|^2
            eb_sbs, enorms = [], []
            for s in range(NSLICE):
                eb_ps = ps_mm.tile([128, E], FP32, tag="mm")
                nc.tensor.transpose(
                    eb_ps[:], emb_aug[0:E, s * 128:(s + 1) * 128].bitcast(FP32),
                    ident[0:E, 0:E])
                eb_sb = stat.tile([128, E], FP32, tag="ebm")
                nc.vector.tensor_copy(eb_sb[:], eb_ps[:])
                nc.sync.dma_start(
                    emb_d[row0 + s * 128:row0 + (s + 1) * 128, :], eb_sb[:])
                en_sq = stat.tile([128, E], FP32, tag="ensq")
                enorm = stat.tile([128, 1], FP32, tag="enorm")
                nc.scalar.activation(en_sq[:], eb_sb[:], AF.Square,
                                     accum_out=enorm[:])
                eb_sbs.append(eb_sb)
                enorms.append(enorm)

            # mm3: h2T[hc] [128, 512] = relu(W3.T @ embT + b3)
            h2T = []
            for hc in range(2):
                h2_ps = ps_mm.tile([128, BF], FP32, tag="mm")
                nc.tensor.matmul(h2_ps[:],
                                 w3_sb[:, hc * 128:(hc + 1) * 128],
                                 emb_aug[0:E, :],
                                 start=True, stop=True)
                h2_sb = actp.tile([128, BF], DEC_DT, tag="h2T")
                nc.scalar.activation(h2_sb[:], h2_ps[:], AF.Relu,
                                     bias=b3_sb[:, hc:hc + 1])
                h2T.append(h2_sb)

            # per 128-row slice: distances + softmin + reconstruction
            for s in range(NSLICE):
                rows = slice(row0 + s * 128, row0 + (s + 1) * 128)
                bsl = slice(s * 128, (s + 1) * 128)

                # shifted distances [128, 512] = [emb;1].T @ [-2 reps^T; |reps|^2]
                d_ps = ps_mm.tile([128, K], FP32, tag="mm")
                nc.tensor.matmul(d_ps[:], emb_aug[:, bsl], reps_aug[:],
                                 start=True, stop=True)
                # true distances = shifted + |emb|^2 (per-partition bias)
                dist_sb = outp.tile([128, K], FP32, tag="dist")
                nc.scalar.activation(dist_sb[:], d_ps[:], AF.Identity,
                                     bias=enorms[s][:])
                nc.sync.dma_start(dist_d[rows, :], dist_sb[:])

                # softmin on the shifted distances (shift-invariant)
                dmin = stat.tile([128, 1], FP32, tag="dmin")
                nc.vector.tensor_reduce(dmin[:], d_ps[:], axis=AX.X, op=ALU.min)
                amin = stat.tile([128, 1], FP32, tag="amin")
                nc.scalar.mul(amin[:], dmin[:], ALPHA)
                expt = outp.tile([128, K], FP32, tag="expt")
                sume = stat.tile([128, 1], FP32, tag="sume")
                nc.scalar.activation(expt[:], d_ps[:], AF.Exp,
                                     bias=amin[:], scale=-ALPHA,
                                     accum_out=sume[:])
                rcp = stat.tile([128, 1], FP32, tag="rcp")
                nc.vector.reciprocal(rcp[:], sume[:])
                wout = outp.tile([128, K], FP32, tag="wout")
                nc.vector.scalar_tensor_tensor(
                    wout[:], expt[:], rcp[:], dist_sb[:],
                    op0=ALU.mult, op1=ALU.mult)
                nc.sync.dma_start(wd_d[rows, :], wout[:])

                # reconstruction [128, 784] = h2 @ W4 + b4
                r_ps0 = ps_mm.tile([128, 512], FP32, tag="mm")
                r_ps1 = ps_mm.tile([128, D - 512], FP32, tag="mm")
                for hc in range(2):
                    nc.tensor.matmul(r_ps0[:], h2T[hc][:, bsl],
                                     w4_sb[:, hc, 0:512],
                                     start=(hc == 0), stop=(hc == 1))
                    nc.tensor.matmul(r_ps1[:], h2T[hc][:, bsl],
                                     w4_sb[:, hc, 512:D],
                                     start=(hc == 0), stop=(hc == 1))
                rec_sb = outp.tile([128, D], FP32, tag="rec")
                nc.vector.tensor_add(rec_sb[:, 0:512], r_ps0[:],
                                     b4_bcast[:, 0:512])
                nc.vector.tensor_add(rec_sb[:, 512:D], r_ps1[:],
                                     b4_bcast[:, 512:D])
                nc.sync.dma_start(rec_d[rows, :], rec_sb[:])

    nc.compile()
    return nc


_NC_CACHE = None


def kernel(x, cluster_reps, W1, b1, W2, b2, W3, b3, W4, b4):
    global _NC_CACHE
    if _NC_CACHE is None:
        _NC_CACHE = build_kernel()
    nc = _NC_CACHE

    x = np.ascontiguousarray(np.asarray(x, dtype=np.float32))
    shared = {
        "cluster_reps": np.ascontiguousarray(np.asarray(cluster_reps, np.float32)),
        "W1": np.ascontiguousarray(np.asarray(W1, np.float32)),
        "b1": np.ascontiguousarray(np.asarray(b1, np.float32)),
        "W2": np.ascontiguousarray(np.asarray(W2, np.float32)),
        "b2": np.ascontiguousarray(np.asarray(b2, np.float32)),
        "W3": np.ascontiguousarray(np.asarray(W3, np.float32)),
        "b3": np.ascontiguousarray(np.asarray(b3, np.float32)),
        "W4": np.ascontiguousarray(np.asarray(W4, np.float32)),
        "b4": np.ascontiguousarray(np.asarray(b4, np.float32)),
    }
    in_maps = [
        {"x": x[c * BL:(c + 1) * BL], **shared} for c in range(N_CORES)
    ]
    res = run_bass_kernel_spmd(nc, in_maps, list(range(N_CORES)))

    weighted = np.concatenate([res.results[c]["weighted"] for c in range(N_CORES)])
    distances = np.concatenate([res.results[c]["distances"] for c in range(N_CORES)])
    reconstruction = np.concatenate(
        [res.results[c]["reconstruction"] for c in range(N_CORES)])
    embeddings = np.concatenate(
        [res.results[c]["embeddings"] for c in range(N_CORES)])
    return (weighted, distances, reconstruction, embeddings)
